# revision 25
# baseline (speedup 1.0000x reference)
"""Additive (Bahdanau) attention on 8 TRN2 NeuronCores — sine-series kernel.

Per batch b (one NeuronCore each):
    qp[q,h] = Q[q,:] @ Wq.T + bq
    kp[k,h] = K[k,:] @ Wk.T + bk + b_param
    E[q,k]  = sum_h v[h] * tanh(qp[q,h] + kp[k,h])
    A = softmax_k(E + mask_penalty); ctx = A @ V

Key trick: tanh(x) ~ sum_t g_t sin(w_t x) (least-squares sine series,
w_t = t*pi/L).  sin(w(q+k)) separates:
    sin(wq)cos(wk) + cos(wq)sin(wk),  cos(z) = 1 - 2 sin^2(z/2)
so with s = sin(wx), u = sin^2(wx/2) per side:
    E = sum_t g_t [ s_q + s_k - 2 s_q u_k - 2 u_q s_k ]
The pure-q term is softmax-invariant and is dropped.  E becomes ONE PE
matmul with contraction over (3 blocks per t) x h:
    blocks per t: (s_q | -2 g v u_k), (u_q | -2 g v s_k), (-0.5 | -2 g v s_k)

Engine mapping per core:
  - projections (PE, bf16), psum->sbuf copy folds biases (DVE)
  - per (t, side): y = x * w/(2pi) (DVE ts), r = round(y) via the
    +-1.5*2^23 magic trick (one fused DVE ts), f = y - r (DVE TT),
    s = ACT Sin(f, scale=2pi), s' = ACT Sin(f, scale=pi),
    u = ACT Square(s'); k-side weighted to bf16 by DVE ts (v col, -2g)
  - energies^T [k, q] accumulate in PSUM over 3T*2 chunk matmuls; the
    first matmul deposits the mask penalty and zeroes the bank
  - exp (ACT, PSUM src) -> bf16; sums via exp-as-weights matmul with a
    ones column; context = exp^T.T @ V with 1/sum as per-partition scale
    on the psum->sbuf copy; attention out via PE transpose + scale.
"""

import numpy as np

B, LQ, LK = 8, 256, 256
D, H = 512, 256
HC, KC, QC, DC = 2, 2, 2, 4
T_FREQ = 6
L_PERIOD = 7.0
RIDGE = 1e-6
XMAX = 5.2

_CACHE: dict = {}


def _fit_sine(T=T_FREQ, L=L_PERIOD, ridge=RIDGE, xmax=XMAX,
              nsamp=200000, seed=0):
    rng = np.random.default_rng(seed)
    xs = np.concatenate([rng.normal(0, 0.85, nsamp),
                         np.linspace(-xmax, xmax, 4001)])
    w = np.concatenate([np.full(nsamp, 1.0),
                        np.full(4001, nsamp / 4001 * 0.05)])
    om = np.arange(1, T + 1) * np.pi / L
    A = np.sin(xs[:, None] * om[None, :])
    Wm = np.sqrt(w)[:, None]
    AtA = (A * Wm).T @ (A * Wm) + ridge * nsamp * np.eye(T)
    Atb = (A * Wm).T @ (np.tanh(xs) * Wm[:, 0])
    g = np.linalg.solve(AtA, Atb)
    return om, g


def _build_nc():
    import concourse.bacc as bacc
    import concourse.tile as tile
    from concourse import mybir

    f32 = mybir.dt.float32
    bf16 = mybir.dt.bfloat16
    i32 = mybir.dt.int32
    AF = mybir.ActivationFunctionType
    ALU = mybir.AluOpType

    om, gam = _fit_sine()
    MAGIC = float(1.5 * 2 ** 23)
    TWO_PI = float(2 * np.pi)
    PI = float(np.pi)

    nc = bacc.Bacc("TRN2", target_bir_lowering=False)

    qt = nc.declare_dram_parameter("qt", [D, LQ], f32, isOutput=False)
    kt = nc.declare_dram_parameter("kt", [D, LK], f32, isOutput=False)
    vv = nc.declare_dram_parameter("v", [LK, D], f32, isOutput=False)
    wqt = nc.declare_dram_parameter("wqt", [D, H], f32, isOutput=False)
    wkt = nc.declare_dram_parameter("wkt", [D, H], f32, isOutput=False)
    bq2 = nc.declare_dram_parameter("bq2", [128, HC], f32, isOutput=False)
    bk2 = nc.declare_dram_parameter("bk2", [128, HC], f32, isOutput=False)
    bp2 = nc.declare_dram_parameter("bp2", [128, HC], f32, isOutput=False)
    vp2 = nc.declare_dram_parameter("vp2", [128, HC], f32, isOutput=False)
    msk = nc.declare_dram_parameter("mask2", [1, LK], i32, isOutput=False)
    idn = nc.declare_dram_parameter("ident", [128, 128], f32, isOutput=False)
    out_ctx = nc.declare_dram_parameter("out_ctx", [LQ, D], f32, isOutput=True)
    out_attn = nc.declare_dram_parameter("out_attn", [LQ, LK], f32,
                                         isOutput=True)

    with tile.TileContext(nc) as tc:
        with (
            tc.tile_pool(name="const", bufs=1) as cpool,
            tc.tile_pool(name="stage", bufs=3) as spool,
            tc.tile_pool(name="feat", bufs=1) as fpool,
            tc.tile_pool(name="ftmp", bufs=4) as tpool,
            tc.tile_pool(name="exp", bufs=2) as epool,
            tc.tile_pool(name="outp", bufs=2) as opool,
            tc.tile_pool(name="psA", bufs=4, space="PSUM") as psA,
            tc.tile_pool(name="psB", bufs=4, space="PSUM") as psB,
        ):
            # ---- loads: per-128-row chunks, spread over the 3 DMA
            # issuers, k-side first so projections pipeline with DMA ----
            issuers = [nc.sync, nc.scalar, nc.gpsimd]
            kt_sb, wkt_sb, qt_sb, wqt_sb = [], [], [], []
            plan = []
            for dc in range(DC):
                plan.append((kt, kt_sb, LK, dc))
                plan.append((wkt, wkt_sb, H, dc))
            for dc in range(DC):
                plan.append((qt, qt_sb, LQ, dc))
                plan.append((wqt, wqt_sb, H, dc))
            for n, (src, dst, w, dc) in enumerate(plan):
                st = spool.tile([128, w], f32, tag=f"ls{n % 6}",
                                name=f"ls_{src.name}{dc}")
                issuers[n % 3].dma_start(st, src[dc * 128:(dc + 1) * 128, :])
                t = cpool.tile([128, w], bf16, tag=f"{src.name}bf{dc}",
                               name=f"{src.name}bf{dc}")
                nc.scalar.activation(t, st, AF.Copy)
                dst.append(t)

            bq_sb = cpool.tile([128, HC], f32, tag="bq")
            nc.gpsimd.dma_start(bq_sb, bq2[:])
            bk_sb = cpool.tile([128, HC], f32, tag="bk")
            nc.gpsimd.dma_start(bk_sb, bk2[:])
            bp_sb = cpool.tile([128, HC], f32, tag="bp")
            nc.gpsimd.dma_start(bp_sb, bp2[:])
            vp_sb = cpool.tile([128, HC], f32, tag="vp")
            nc.gpsimd.dma_start(vp_sb, vp2[:])
            bkb = cpool.tile([128, HC], f32, tag="bkb")
            nc.vector.tensor_add(bkb, bk_sb, bp_sb)
            msk_sb = cpool.tile([1, LK], i32, tag="msk")
            nc.sync.dma_start(msk_sb, msk[:])
            mask_bf = cpool.tile([1, LK], bf16, tag="maskbf")
            nc.vector.tensor_scalar(mask_bf, msk_sb, 0, -1e30, ALU.is_equal,
                                    ALU.mult)
            ones_row = cpool.tile([1, LQ], bf16, tag="onesrow")
            nc.vector.memset(ones_row, 1.0)
            # q-side "ones" feature carries the -0.5 factor
            halfneg = cpool.tile([128, LQ], bf16, tag="halfneg")
            nc.vector.memset(halfneg, -0.5)

            # ---- projections into one [q|k]-concat tile [128,(side,hc,n)] ----
            xcat = cpool.tile([128, 2, HC, LQ], f32, tag="xcat")
            for hc in range(HC):
                pk = psA.tile([128, LK], f32, tag="ps")
                for dc in range(DC):
                    nc.tensor.matmul(
                        pk, lhsT=wkt_sb[dc][:, hc * 128:(hc + 1) * 128],
                        rhs=kt_sb[dc], start=(dc == 0), stop=(dc == DC - 1))
                nc.vector.tensor_scalar_add(xcat[:, 1, hc, :], pk,
                                            bkb[:, hc:hc + 1])
                pq = psA.tile([128, LQ], f32, tag="ps")
                for dc in range(DC):
                    nc.tensor.matmul(
                        pq, lhsT=wqt_sb[dc][:, hc * 128:(hc + 1) * 128],
                        rhs=qt_sb[dc], start=(dc == 0), stop=(dc == DC - 1))
                nc.vector.tensor_scalar_add(xcat[:, 0, hc, :], pq,
                                            bq_sb[:, hc:hc + 1])

            # late-needed tensors: V, identity (after feature chain kickoff)
            v_bf = []
            for kc in range(KC):
                vf = spool.tile([128, D], f32, tag="vstage")
                nc.gpsimd.dma_start(vf, vv[kc * 128:(kc + 1) * 128, :])
                vb = cpool.tile([128, D], bf16, tag=f"v{kc}")
                nc.vector.tensor_copy(vb, vf)
                v_bf.append(vb)
            idf = spool.tile([128, 128], f32, tag="idstage")
            nc.sync.dma_start(idf, idn[:])
            id_bf = cpool.tile([128, 128], bf16, tag="idbf")
            nc.vector.tensor_copy(id_bf, idf)
            ones_col = cpool.tile([128, 1], bf16, tag="ones")
            nc.vector.memset(ones_col, 1.0)

            # ---- energies^T psum tiles [k, q], one per k-chunk ----
            et = [psA.tile([128, LQ], f32, tag="ps", name=f"et{kc}")
                  for kc in range(KC)]
            for kc in range(KC):
                nc.tensor.matmul(et[kc],
                                 lhsT=mask_bf[:, kc * 128:(kc + 1) * 128],
                                 rhs=ones_row, start=True, stop=False)

            # ---- per-frequency features + energy matmuls ----
            n_mm = [1, 1]   # per-kc matmul count (mask mm counted)
            total_mm = 1 + T_FREQ * 3 * HC
            for t in range(T_FREQ):
                sc_y = float(om[t] / TWO_PI)
                g = float(gam[t])
                sides = {}
                for side in (1, 0):    # k-side first
                    y = tpool.tile([128, HC, 256], f32, tag=f"y{side}",
                                   name=f"y{side}_{t}")
                    nc.vector.tensor_scalar(y, xcat[:, side], sc_y, None,
                                            ALU.mult)
                    r = tpool.tile([128, HC, 256], f32, tag=f"r{side}",
                                   name=f"r{side}_{t}")
                    nc.vector.tensor_scalar(r, y, MAGIC, MAGIC, ALU.add,
                                            ALU.subtract)
                    f = tpool.tile([128, HC, 256], f32, tag=f"f{side}",
                                   name=f"f{side}_{t}")
                    nc.vector.tensor_sub(f, y, r)
                    s_t = fpool.tile([128, HC, 256], bf16, tag=f"s{side}_{t}",
                                     name=f"s{side}_{t}")
                    nc.scalar.activation(s_t, f, AF.Sin, scale=TWO_PI)
                    sp = tpool.tile([128, HC, 256], f32, tag=f"sp{side}",
                                    name=f"sp{side}_{t}")
                    nc.scalar.activation(sp, f, AF.Sin, scale=PI)
                    u_t = fpool.tile([128, HC, 256], bf16, tag=f"u{side}_{t}",
                                     name=f"u{side}_{t}")
                    nc.scalar.activation(u_t, sp, AF.Square)
                    sides[side] = (s_t, u_t)

                # k-side weighted: W_s = -2 g v s_k, W_u = -2 g v u_k
                ws = fpool.tile([128, HC, 256], bf16, tag=f"ws{t}",
                                name=f"ws{t}")
                wu = fpool.tile([128, HC, 256], bf16, tag=f"wu{t}",
                                name=f"wu{t}")
                for hc in range(HC):
                    nc.vector.tensor_scalar(
                        ws[:, hc, :], sides[1][0][:, hc, :],
                        vp_sb[:, hc:hc + 1], -2.0 * g, ALU.mult, ALU.mult)
                    nc.vector.tensor_scalar(
                        wu[:, hc, :], sides[1][1][:, hc, :],
                        vp_sb[:, hc:hc + 1], -2.0 * g, ALU.mult, ALU.mult)

                s_q, u_q = sides[0]
                for kc in range(KC):
                    for wf, qview in ((wu, s_q), (ws, u_q), (ws, None)):
                        for hc in range(HC):
                            n_mm[kc] += 1
                            rhs = (halfneg if qview is None
                                   else qview[:, hc, :])
                            nc.tensor.matmul(
                                et[kc],
                                lhsT=wf[:, hc, kc * 128:(kc + 1) * 128],
                                rhs=rhs,
                                start=False,
                                stop=(n_mm[kc] == total_mm))

            # ---- softmax + context + attention out ----
            expts = []
            for kc in range(KC):
                e = epool.tile([128, LQ], bf16, tag="exp", name=f"exp{kc}")
                nc.scalar.activation(e, et[kc], AF.Exp)
                expts.append(e)

            for qc in range(QC):
                sums = psB.tile([128, 1], f32, tag="misc", name=f"sums{qc}")
                for kc in range(KC):
                    nc.tensor.matmul(
                        sums, lhsT=expts[kc][:, qc * 128:(qc + 1) * 128],
                        rhs=ones_col, start=(kc == 0), stop=(kc == KC - 1))
                recip = opool.tile([128, 1], f32, tag="recip",
                                   name=f"recip{qc}")
                nc.vector.reciprocal(recip, sums)

                ctxp = psB.tile([128, D], f32, tag="misc", name=f"ctxp{qc}")
                for kc in range(KC):
                    nc.tensor.matmul(
                        ctxp, lhsT=expts[kc][:, qc * 128:(qc + 1) * 128],
                        rhs=v_bf[kc], start=(kc == 0), stop=(kc == KC - 1))
                ctx_sb = opool.tile([128, D], f32, tag="ctx",
                                    name=f"ctx{qc}")
                nc.vector.tensor_scalar_mul(ctx_sb, ctxp, recip)
                nc.sync.dma_start(out_ctx[qc * 128:(qc + 1) * 128, :], ctx_sb)

                attn_sb = opool.tile([128, LK], f32, tag="attn",
                                     name=f"attn{qc}")
                for kc in range(KC):
                    tp = psB.tile([128, 128], bf16, tag="misc",
                                  name=f"tp{qc}{kc}")
                    nc.tensor.transpose(
                        tp, expts[kc][:, qc * 128:(qc + 1) * 128], id_bf)
                    nc.vector.tensor_scalar_mul(
                        attn_sb[:, kc * 128:(kc + 1) * 128], tp, recip)
                nc.sync.dma_start(out_attn[qc * 128:(qc + 1) * 128, :],
                                  attn_sb)

    nc.compile()
    return nc


def _get_nc():
    if "nc" not in _CACHE:
        _CACHE["nc"] = _build_nc()
    return _CACHE["nc"]


def make_in_maps(Q, K, V, mask, Wq, bq, Wk, bk, v_param, b_param):
    Q = np.asarray(Q, dtype=np.float32)
    K = np.asarray(K, dtype=np.float32)
    V = np.asarray(V, dtype=np.float32)
    mask = np.asarray(mask, dtype=np.int32)
    Wq = np.asarray(Wq, dtype=np.float32)
    Wk = np.asarray(Wk, dtype=np.float32)
    bq = np.asarray(bq, dtype=np.float32)
    bk = np.asarray(bk, dtype=np.float32)
    v_param = np.asarray(v_param, dtype=np.float32)
    b_param = np.asarray(b_param, dtype=np.float32)

    wqt = np.ascontiguousarray(Wq.T)
    wkt = np.ascontiguousarray(Wk.T)
    bq2 = np.ascontiguousarray(bq.reshape(HC, 128).T)
    bk2 = np.ascontiguousarray(bk.reshape(HC, 128).T)
    bp2 = np.ascontiguousarray(b_param.reshape(HC, 128).T)
    vp2 = np.ascontiguousarray(v_param.reshape(HC, 128).T)
    ident = np.eye(128, dtype=np.float32)

    in_maps = []
    for b in range(B):
        in_maps.append({
            "qt": np.ascontiguousarray(Q[b].T),
            "kt": np.ascontiguousarray(K[b].T),
            "v": np.ascontiguousarray(V[b]),
            "wqt": wqt,
            "wkt": wkt,
            "bq2": bq2,
            "bk2": bk2,
            "bp2": bp2,
            "vp2": vp2,
            "mask2": np.ascontiguousarray(mask[b].reshape(1, LK)),
            "ident": ident,
        })
    return in_maps


def kernel(Q, K, V, mask, Wq, bq, Wk, bk, v_param, b_param, _trace=False):
    from concourse.bass_utils import run_bass_kernel_spmd

    nc = _get_nc()
    in_maps = make_in_maps(Q, K, V, mask, Wq, bq, Wk, bk, v_param, b_param)
    res = run_bass_kernel_spmd(nc, in_maps, core_ids=list(range(B)),
                               trace=_trace)
    outs = res.results
    context = np.stack([np.asarray(outs[b]["out_ctx"]) for b in range(B)])
    attn = np.stack([np.asarray(outs[b]["out_attn"]) for b in range(B)])
    if _trace:
        return (context, attn), res
    return context, attn


# revision 26
# speedup vs baseline: 1.2036x; 1.2036x over previous
"""Additive (Bahdanau) attention on 8 TRN2 NeuronCores — sine-series kernel.

Per batch b (one NeuronCore each):
    qp[q,h] = Q[q,:] @ Wq.T + bq
    kp[k,h] = K[k,:] @ Wk.T + bk + b_param
    E[q,k]  = sum_h v[h] * tanh(qp[q,h] + kp[k,h])
    A = softmax_k(E + mask_penalty); ctx = A @ V

Key trick: tanh(x) ~ sum_t g_t sin(w_t x) (least-squares sine series,
w_t = t*pi/L).  sin(w(q+k)) separates:
    sin(wq)cos(wk) + cos(wq)sin(wk),  cos(z) = 1 - 2 sin^2(z/2)
so with s = sin(wx), u = sin^2(wx/2) per side:
    E = sum_t g_t [ s_q + s_k - 2 s_q u_k - 2 u_q s_k ]
The pure-q term is softmax-invariant and is dropped.  E becomes ONE PE
matmul with contraction over (3 blocks per t) x h:
    blocks per t: (s_q | -2 g v u_k), (u_q | -2 g v s_k), (-0.5 | -2 g v s_k)

Engine mapping per core:
  - projections (PE, bf16), psum->sbuf copy folds biases (DVE)
  - per (t, side): y = x * w/(2pi) (DVE ts), r = round(y) via the
    +-1.5*2^23 magic trick (one fused DVE ts), f = y - r (DVE TT),
    s = ACT Sin(f, scale=2pi), s' = ACT Sin(f, scale=pi),
    u = ACT Square(s'); k-side weighted to bf16 by DVE ts (v col, -2g)
  - energies^T [k, q] accumulate in PSUM over 3T*2 chunk matmuls; the
    first matmul deposits the mask penalty and zeroes the bank
  - exp (ACT, PSUM src) -> bf16; sums via exp-as-weights matmul with a
    ones column; context = exp^T.T @ V with 1/sum as per-partition scale
    on the psum->sbuf copy; attention out via PE transpose + scale.
"""

import numpy as np

B, LQ, LK = 8, 256, 256
D, H = 512, 256
HC, KC, QC, DC = 2, 2, 2, 4
T_FREQ = 6
L_PERIOD = 7.0
RIDGE = 1e-6
XMAX = 5.2

_CACHE: dict = {}


def _fit_sine(T=T_FREQ, L=L_PERIOD, ridge=RIDGE, xmax=XMAX,
              nsamp=200000, seed=0):
    rng = np.random.default_rng(seed)
    xs = np.concatenate([rng.normal(0, 0.85, nsamp),
                         np.linspace(-xmax, xmax, 4001)])
    w = np.concatenate([np.full(nsamp, 1.0),
                        np.full(4001, nsamp / 4001 * 0.05)])
    om = np.arange(1, T + 1) * np.pi / L
    A = np.sin(xs[:, None] * om[None, :])
    Wm = np.sqrt(w)[:, None]
    AtA = (A * Wm).T @ (A * Wm) + ridge * nsamp * np.eye(T)
    Atb = (A * Wm).T @ (np.tanh(xs) * Wm[:, 0])
    g = np.linalg.solve(AtA, Atb)
    return om, g


def _build_nc():
    import concourse.bacc as bacc
    import concourse.tile as tile
    from concourse import mybir

    f32 = mybir.dt.float32
    bf16 = mybir.dt.bfloat16
    i32 = mybir.dt.int32
    AF = mybir.ActivationFunctionType
    ALU = mybir.AluOpType

    om, gam = _fit_sine()
    MAGIC = float(1.5 * 2 ** 23)
    TWO_PI = float(2 * np.pi)
    PI = float(np.pi)

    nc = bacc.Bacc("TRN2", target_bir_lowering=False)

    qt = nc.declare_dram_parameter("qt", [D, LQ], f32, isOutput=False)
    kt = nc.declare_dram_parameter("kt", [D, LK], f32, isOutput=False)
    vv = nc.declare_dram_parameter("v", [LK, D], f32, isOutput=False)
    wqt = nc.declare_dram_parameter("wqt", [D, H], f32, isOutput=False)
    wkt = nc.declare_dram_parameter("wkt", [D, H], f32, isOutput=False)
    bq2 = nc.declare_dram_parameter("bq2", [128, HC], f32, isOutput=False)
    bk2 = nc.declare_dram_parameter("bk2", [128, HC], f32, isOutput=False)
    bp2 = nc.declare_dram_parameter("bp2", [128, HC], f32, isOutput=False)
    vp2 = nc.declare_dram_parameter("vp2", [128, HC], f32, isOutput=False)
    msk = nc.declare_dram_parameter("mask2", [1, LK], i32, isOutput=False)
    idn = nc.declare_dram_parameter("ident", [128, 128], f32, isOutput=False)
    out_ctx = nc.declare_dram_parameter("out_ctx", [LQ, D], f32, isOutput=True)
    out_attn = nc.declare_dram_parameter("out_attn", [LQ, LK], f32,
                                         isOutput=True)

    with tile.TileContext(nc) as tc:
        with (
            tc.tile_pool(name="const", bufs=1) as cpool,
            tc.tile_pool(name="stage", bufs=3) as spool,
            tc.tile_pool(name="feat", bufs=1) as fpool,
            tc.tile_pool(name="ftmp", bufs=4) as tpool,
            tc.tile_pool(name="exp", bufs=2) as epool,
            tc.tile_pool(name="outp", bufs=2) as opool,
            tc.tile_pool(name="psA", bufs=4, space="PSUM") as psA,
            tc.tile_pool(name="psB", bufs=4, space="PSUM") as psB,
        ):
            # ---- loads: per-128-row chunks, spread over the 3 DMA
            # issuers, k-side first so projections pipeline with DMA ----
            bq_sb = cpool.tile([128, HC], f32, tag="bq")
            nc.gpsimd.dma_start(bq_sb, bq2[:])
            bk_sb = cpool.tile([128, HC], f32, tag="bk")
            nc.gpsimd.dma_start(bk_sb, bk2[:])
            bp_sb = cpool.tile([128, HC], f32, tag="bp")
            nc.gpsimd.dma_start(bp_sb, bp2[:])
            vp_sb = cpool.tile([128, HC], f32, tag="vp")
            nc.gpsimd.dma_start(vp_sb, vp2[:])
            msk_sb = cpool.tile([1, LK], i32, tag="msk")
            nc.gpsimd.dma_start(msk_sb, msk[:])

            issuers = [nc.sync, nc.scalar]
            kt_sb, wkt_sb, qt_sb, wqt_sb = [], [], [], []
            plan = []
            for dc in range(DC):
                plan.append((kt, kt_sb, LK, dc))
                plan.append((wkt, wkt_sb, H, dc))
            for dc in range(DC):
                plan.append((qt, qt_sb, LQ, dc))
                plan.append((wqt, wqt_sb, H, dc))
            for n, (src, dst, w, dc) in enumerate(plan):
                st = spool.tile([128, w], f32, tag=f"ls{n % 6}",
                                name=f"ls_{src.name}{dc}")
                issuers[n % 2].dma_start(st, src[dc * 128:(dc + 1) * 128, :])
                t = cpool.tile([128, w], bf16, tag=f"{src.name}bf{dc}",
                               name=f"{src.name}bf{dc}")
                nc.scalar.activation(t, st, AF.Copy)
                dst.append(t)

            bkb = cpool.tile([128, HC], f32, tag="bkb")
            nc.vector.tensor_add(bkb, bk_sb, bp_sb)
            mask_bf = cpool.tile([1, LK], bf16, tag="maskbf")
            nc.vector.tensor_scalar(mask_bf, msk_sb, 0, -1e30, ALU.is_equal,
                                    ALU.mult)
            ones_row = cpool.tile([1, LQ], bf16, tag="onesrow")
            nc.vector.memset(ones_row, 1.0)
            # q-side "ones" feature carries the -0.5 factor
            halfneg = cpool.tile([128, LQ], bf16, tag="halfneg")
            nc.vector.memset(halfneg, -0.5)

            # ---- projections into one [q|k]-concat tile [128,(side,hc,n)] ----
            xcat = cpool.tile([128, 2, HC, LQ], f32, tag="xcat")
            for hc in range(HC):
                pk = psA.tile([128, LK], f32, tag="ps")
                for dc in range(DC):
                    nc.tensor.matmul(
                        pk, lhsT=wkt_sb[dc][:, hc * 128:(hc + 1) * 128],
                        rhs=kt_sb[dc], start=(dc == 0), stop=(dc == DC - 1))
                nc.vector.tensor_scalar_add(xcat[:, 1, hc, :], pk,
                                            bkb[:, hc:hc + 1])
                pq = psA.tile([128, LQ], f32, tag="ps")
                for dc in range(DC):
                    nc.tensor.matmul(
                        pq, lhsT=wqt_sb[dc][:, hc * 128:(hc + 1) * 128],
                        rhs=qt_sb[dc], start=(dc == 0), stop=(dc == DC - 1))
                nc.vector.tensor_scalar_add(xcat[:, 0, hc, :], pq,
                                            bq_sb[:, hc:hc + 1])

            # late-needed tensors: V, identity (after feature chain kickoff)
            v_bf = []
            for kc in range(KC):
                vf = spool.tile([128, D], f32, tag="vstage")
                nc.gpsimd.dma_start(vf, vv[kc * 128:(kc + 1) * 128, :])
                vb = cpool.tile([128, D], bf16, tag=f"v{kc}")
                nc.vector.tensor_copy(vb, vf)
                v_bf.append(vb)
            idf = spool.tile([128, 128], f32, tag="idstage")
            nc.sync.dma_start(idf, idn[:])
            id_bf = cpool.tile([128, 128], bf16, tag="idbf")
            nc.vector.tensor_copy(id_bf, idf)
            ones_col = cpool.tile([128, 1], bf16, tag="ones")
            nc.vector.memset(ones_col, 1.0)

            # ---- energies^T psum tiles [k, q], one per k-chunk ----
            et = [psA.tile([128, LQ], f32, tag="ps", name=f"et{kc}")
                  for kc in range(KC)]
            for kc in range(KC):
                nc.tensor.matmul(et[kc],
                                 lhsT=mask_bf[:, kc * 128:(kc + 1) * 128],
                                 rhs=ones_row, start=True, stop=False)

            # ---- per-frequency features + energy matmuls ----
            n_mm = [1, 1]   # per-kc matmul count (mask mm counted)
            total_mm = 1 + T_FREQ * 3 * HC
            for t in range(T_FREQ):
                sc_y = float(om[t] / TWO_PI)
                g = float(gam[t])
                sides = {}
                for side in (1, 0):    # k-side first
                    y = tpool.tile([128, HC, 256], f32, tag=f"y{side}",
                                   name=f"y{side}_{t}")
                    nc.vector.tensor_scalar(y, xcat[:, side], sc_y, None,
                                            ALU.mult)
                    r = tpool.tile([128, HC, 256], f32, tag=f"r{side}",
                                   name=f"r{side}_{t}")
                    nc.vector.tensor_scalar(r, y, MAGIC, MAGIC, ALU.add,
                                            ALU.subtract)
                    f = tpool.tile([128, HC, 256], f32, tag=f"f{side}",
                                   name=f"f{side}_{t}")
                    nc.vector.tensor_sub(f, y, r)
                    s_t = fpool.tile([128, HC, 256], bf16, tag=f"s{side}_{t}",
                                     name=f"s{side}_{t}")
                    nc.scalar.activation(s_t, f, AF.Sin, scale=TWO_PI)
                    sp = tpool.tile([128, HC, 256], f32, tag=f"sp{side}",
                                    name=f"sp{side}_{t}")
                    nc.scalar.activation(sp, f, AF.Sin, scale=PI)
                    u_t = fpool.tile([128, HC, 256], bf16, tag=f"u{side}_{t}",
                                     name=f"u{side}_{t}")
                    nc.scalar.activation(u_t, sp, AF.Square)
                    sides[side] = (s_t, u_t)

                # k-side weighted: W_s = -2 g v s_k, W_u = -2 g v u_k
                ws = fpool.tile([128, HC, 256], bf16, tag=f"ws{t}",
                                name=f"ws{t}")
                wu = fpool.tile([128, HC, 256], bf16, tag=f"wu{t}",
                                name=f"wu{t}")
                for hc in range(HC):
                    nc.vector.tensor_scalar(
                        ws[:, hc, :], sides[1][0][:, hc, :],
                        vp_sb[:, hc:hc + 1], -2.0 * g, ALU.mult, ALU.mult)
                    nc.vector.tensor_scalar(
                        wu[:, hc, :], sides[1][1][:, hc, :],
                        vp_sb[:, hc:hc + 1], -2.0 * g, ALU.mult, ALU.mult)

                s_q, u_q = sides[0]
                for kc in range(KC):
                    for wf, qview in ((wu, s_q), (ws, u_q), (ws, None)):
                        for hc in range(HC):
                            n_mm[kc] += 1
                            rhs = (halfneg if qview is None
                                   else qview[:, hc, :])
                            nc.tensor.matmul(
                                et[kc],
                                lhsT=wf[:, hc, kc * 128:(kc + 1) * 128],
                                rhs=rhs,
                                start=False,
                                stop=(n_mm[kc] == total_mm))

            # ---- softmax + context + attention out ----
            expts = []
            for kc in range(KC):
                e = epool.tile([128, LQ], bf16, tag="exp", name=f"exp{kc}")
                nc.scalar.activation(e, et[kc], AF.Exp)
                expts.append(e)

            for qc in range(QC):
                sums = psB.tile([128, 1], f32, tag="misc", name=f"sums{qc}")
                for kc in range(KC):
                    nc.tensor.matmul(
                        sums, lhsT=expts[kc][:, qc * 128:(qc + 1) * 128],
                        rhs=ones_col, start=(kc == 0), stop=(kc == KC - 1))
                recip = opool.tile([128, 1], f32, tag="recip",
                                   name=f"recip{qc}")
                nc.vector.reciprocal(recip, sums)

                ctxp = psB.tile([128, D], f32, tag="misc", name=f"ctxp{qc}")
                for kc in range(KC):
                    nc.tensor.matmul(
                        ctxp, lhsT=expts[kc][:, qc * 128:(qc + 1) * 128],
                        rhs=v_bf[kc], start=(kc == 0), stop=(kc == KC - 1))
                ctx_sb = opool.tile([128, D], f32, tag="ctx",
                                    name=f"ctx{qc}")
                nc.vector.tensor_scalar_mul(ctx_sb, ctxp, recip)
                nc.sync.dma_start(out_ctx[qc * 128:(qc + 1) * 128, :], ctx_sb)

                attn_sb = opool.tile([128, LK], f32, tag="attn",
                                     name=f"attn{qc}")
                for kc in range(KC):
                    tp = psB.tile([128, 128], bf16, tag="misc",
                                  name=f"tp{qc}{kc}")
                    nc.tensor.transpose(
                        tp, expts[kc][:, qc * 128:(qc + 1) * 128], id_bf)
                    nc.vector.tensor_scalar_mul(
                        attn_sb[:, kc * 128:(kc + 1) * 128], tp, recip)
                nc.sync.dma_start(out_attn[qc * 128:(qc + 1) * 128, :],
                                  attn_sb)

    nc.compile()
    return nc


def _get_nc():
    if "nc" not in _CACHE:
        _CACHE["nc"] = _build_nc()
    return _CACHE["nc"]


def make_in_maps(Q, K, V, mask, Wq, bq, Wk, bk, v_param, b_param):
    Q = np.asarray(Q, dtype=np.float32)
    K = np.asarray(K, dtype=np.float32)
    V = np.asarray(V, dtype=np.float32)
    mask = np.asarray(mask, dtype=np.int32)
    Wq = np.asarray(Wq, dtype=np.float32)
    Wk = np.asarray(Wk, dtype=np.float32)
    bq = np.asarray(bq, dtype=np.float32)
    bk = np.asarray(bk, dtype=np.float32)
    v_param = np.asarray(v_param, dtype=np.float32)
    b_param = np.asarray(b_param, dtype=np.float32)

    wqt = np.ascontiguousarray(Wq.T)
    wkt = np.ascontiguousarray(Wk.T)
    bq2 = np.ascontiguousarray(bq.reshape(HC, 128).T)
    bk2 = np.ascontiguousarray(bk.reshape(HC, 128).T)
    bp2 = np.ascontiguousarray(b_param.reshape(HC, 128).T)
    vp2 = np.ascontiguousarray(v_param.reshape(HC, 128).T)
    ident = np.eye(128, dtype=np.float32)

    in_maps = []
    for b in range(B):
        in_maps.append({
            "qt": np.ascontiguousarray(Q[b].T),
            "kt": np.ascontiguousarray(K[b].T),
            "v": np.ascontiguousarray(V[b]),
            "wqt": wqt,
            "wkt": wkt,
            "bq2": bq2,
            "bk2": bk2,
            "bp2": bp2,
            "vp2": vp2,
            "mask2": np.ascontiguousarray(mask[b].reshape(1, LK)),
            "ident": ident,
        })
    return in_maps


def kernel(Q, K, V, mask, Wq, bq, Wk, bk, v_param, b_param, _trace=False):
    from concourse.bass_utils import run_bass_kernel_spmd

    nc = _get_nc()
    in_maps = make_in_maps(Q, K, V, mask, Wq, bq, Wk, bk, v_param, b_param)
    res = run_bass_kernel_spmd(nc, in_maps, core_ids=list(range(B)),
                               trace=_trace)
    outs = res.results
    context = np.stack([np.asarray(outs[b]["out_ctx"]) for b in range(B)])
    attn = np.stack([np.asarray(outs[b]["out_attn"]) for b in range(B)])
    if _trace:
        return (context, attn), res
    return context, attn
